# revision 1
# baseline (speedup 1.0000x reference)
"""Causal self-attention (GQA + RoPE) Trainium2 Bass kernel, 8-core SPMD.

Problem shapes (hardcoded): B=2, T=2048, C=2048, NH=16, NKV=4, HD=128.

Sharding: 8 cores = (batch b in {0,1}) x (kv-group g in {0..3}).
Core c = b*4+g handles batch b, q-heads 4g..4g+3, kv-head g.
  - Wq column-parallel (512 cols/core), Wk/Wv column-parallel (128 cols/core),
    Wproj row-parallel (512 rows/core) -> per-core partial [T, C] outputs,
    host sums the 4 partials per batch.

Per-core dataflow ("transposed flash"):
  - Inputs pre-rearranged on host so every weight/x load is one big DMA
    with contiguous per-partition lines.
  - Projections computed in transposed layout: qT/kT [HD, T]
    (lhsT = W chunk, rhs = xT chunk), V transposed to [T, HD] via PE.
  - RoPE on qT/kT via partition-shifted SBUF copies + cos/sin tables.
  - Scores computed transposed: S^T[tk, tq] = matmul(lhsT=kT block, rhs=qT);
    causal mask applied by preloading PSUM with a -1e5 table via an
    identity matmul, so exp is a single activation per block.
  - Phase B software-pipelined: S for block j+2 issued before PV_j so the
    PE never waits on the scalar-engine exp.
  - exp without max subtraction (scores are O(5) here; safe in fp32);
    row sums L[tq] via ones-vector matmul; reciprocal on the [1,W] row,
    then broadcast via a DRAM-bounce DMA; normalization on DVE.
  - Wproj row-parallel partials; out-proj for tq group g-1 interleaved
    after attention group g to fill pipeline bubbles.
"""

import numpy as np

import concourse.bass as bass
import concourse.bacc as bacc
import concourse.mybir as mybir
import concourse.tile as tile

B, T, C = 2, 2048, 2048
NH, NKV, HD = 16, 4, 128
P = 128
W = 512            # wide tile (PSUM bank = 512 fp32)
TB = T // P        # 16 t blocks
CB = C // P        # 16 c chunks
G = T // W         # 4 tq groups
NQ = 4             # q heads per core

F32 = mybir.dt.float32

USE_F32R = False
MM = mybir.dt.bfloat16    # matmul-input compute dtype


def build_nc():
    nc = bacc.Bacc("TRN2", target_bir_lowering=False)
    xR = nc.dram_tensor("xR", (P, G * CB * W), MM, kind="ExternalInput")[:]
    wqr = nc.dram_tensor("wqr", (P, CB * NQ * HD), MM, kind="ExternalInput")[:]
    wkr = nc.dram_tensor("wkr", (P, CB * HD), MM, kind="ExternalInput")[:]
    wvr = nc.dram_tensor("wvr", (P, CB * HD), MM, kind="ExternalInput")[:]
    wpr = nc.dram_tensor("wpr", (P, NQ * C), MM, kind="ExternalInput")[:]
    cosT = nc.dram_tensor("cosT", (P, T), MM, kind="ExternalInput")[:]
    msinT = nc.dram_tensor("msinT", (P, T), MM, kind="ExternalInput")[:]
    masktb = nc.dram_tensor("masktb", (P, W), MM, kind="ExternalInput")[:]
    ident = nc.dram_tensor("ident", (P, P), MM, kind="ExternalInput")[:]
    onescol = nc.dram_tensor("onescol", (P, 1), MM, kind="ExternalInput")[:]
    out = nc.dram_tensor("out", (T, C), F32, kind="ExternalOutput")[:]

    EXP = mybir.ActivationFunctionType.Exp
    LN = mybir.ActivationFunctionType.Ln

    xr4 = xR.rearrange("p (t cb w) -> p t cb w", t=G, cb=CB)
    wq3 = wqr.rearrange("p (cb m) -> p cb m", cb=CB)
    wk3 = wkr.rearrange("p (cb m) -> p cb m", cb=CB)
    wv3 = wvr.rearrange("p (cb m) -> p cb m", cb=CB)
    wp3 = wpr.rearrange("p (hb c) -> p hb c", hb=NQ)

    with tile.TileContext(nc) as tc:
        with (
            tc.tile_pool(name="singles", bufs=1) as singles,
            tc.tile_pool(name="xin", bufs=2) as xin,
            tc.tile_pool(name="stage", bufs=3) as stage,
            tc.tile_pool(name="ptp", bufs=4) as ptp,
            tc.tile_pool(name="outp", bufs=2) as outp,
            tc.tile_pool(name="small", bufs=2) as small,
            tc.tile_pool(name="dramp", bufs=4, space="DRAM") as dramp,
        ):
            # ---- resident tiles ----
            qT = singles.tile([P, NQ, T], MM)       # roped q
            yT = singles.tile([P, NQ, T], MM)       # attention out (pre-proj)
            kT = singles.tile([P, T], MM)           # roped k, [hd, t]
            Vt = singles.tile([P, TB, HD], MM)      # [t_in_blk, blk, hd]
            cos_s = singles.tile([P, T], MM)
            msin_s = singles.tile([P, T], MM)
            mask_s = singles.tile([P, W], MM)
            id_s = singles.tile([P, P], MM)
            ones_s = singles.tile([P, 1], MM)       # column of ones (lhsT)
            # wq split into 4 sub-tiles so the first matmuls only wait on
            # the first quarter of the weight stream.
            wqp = [singles.tile([P, 4, NQ * HD], MM, name=f"wqp{i}")
                   for i in range(4)]
            wkall = singles.tile([P, CB, HD], MM)
            wvall = singles.tile([P, CB, HD], MM)
            wpall = singles.tile([P, NQ, C], MM)

            # weights + tables on the sync queue; x loads go on the
            # scalar queue so the first x tile races the weight loads.
            for i in range(4):
                nc.sync.dma_start(out=wqp[i], in_=wq3[:, 4 * i:4 * i + 4, :])
            nc.sync.dma_start(out=id_s, in_=ident)
            nc.sync.dma_start(out=cos_s, in_=cosT)
            nc.sync.dma_start(out=msin_s, in_=msinT)
            nc.sync.dma_start(out=wkall, in_=wk3)
            nc.sync.dma_start(out=wvall, in_=wv3)
            nc.sync.dma_start(out=mask_s, in_=masktb)
            nc.sync.dma_start(out=ones_s, in_=onescol)

            def rope_apply(dst, praw, tsl):
                # dst[d,:] = praw[d,:]*cos[d,:] + rot(praw)[d,:]*msin[d,:]
                # rot swaps halves; the rotate-half sign is folded into msin.
                tmp = stage.tile([P, W], F32, tag="ropetmp")
                nc.gpsimd.dma_start(out=tmp[0:64, :], in_=praw[64:128, :])
                nc.gpsimd.dma_start(out=tmp[64:128, :], in_=praw[0:64, :])
                nc.vector.tensor_mul(out=dst, in0=praw, in1=cos_s[:, tsl])
                nc.vector.tensor_mul(out=tmp, in0=tmp, in1=msin_s[:, tsl])
                nc.vector.tensor_add(out=dst, in0=dst, in1=tmp)

            # ---- phase A: QKV projections + rope + V transpose ----
            def lhs_a(m, c):
                if m < 4:
                    return wqp[c // 4][:, c % 4, m * P:(m + 1) * P]
                w = wkall if m == 4 else wvall
                return w[:, c, 0:P]

            with tc.tile_pool(name="pa", bufs=1, space="PSUM") as pa:
                # All 4 x-group loads issued up front on four different
                # engine queues so they stream concurrently from t=0.
                xts = [xin.tile([P, CB, W], MM, tag="x", bufs=4,
                                name=f"x_t{t}") for t in range(G)]
                # t=0 in contiguous quarters: the first matmuls wait only
                # on the first quarter.
                for qq in range(4):
                    nc.scalar.dma_start(out=xts[0][:, 4 * qq:4 * qq + 4, :],
                                        in_=xr4[:, 0, 4 * qq:4 * qq + 4, :])
                # x1 leads the gpsimd queue (done before the first rope
                # swaps need it); x2 behind x0 on scalar; x3 + wp trail
                # the tables on sync.
                nc.gpsimd.dma_start(out=xts[1], in_=xr4[:, 1, :, :])
                nc.scalar.dma_start(out=xts[2], in_=xr4[:, 2, :, :])
                nc.sync.dma_start(out=xts[3], in_=xr4[:, 3, :, :])
                # wp only needed in phase C; load it after everything else
                nc.sync.dma_start(out=wpall, in_=wp3)
                for t in range(G):
                    _phase_a_group(nc, tc, pa, stage, xts[t], t,
                                   lhs_a, qT, kT, Vt, id_s, rope_apply)

            # ---- phases B+C psum pools (A's pool released above) ----
            import contextlib
            ctx_bc = contextlib.ExitStack()
            pmm = ctx_bc.enter_context(
                tc.tile_pool(name="pmm", bufs=1, space="PSUM"))
            pacc = ctx_bc.enter_context(
                tc.tile_pool(name="pacc", bufs=1, space="PSUM"))
            plps = ctx_bc.enter_context(
                tc.tile_pool(name="plps", bufs=1, space="PSUM"))

            def emit_S(g, h, j, gsl):
                """Score matmul for tk block j (with causal-mask preload on
                the diagonal chunk); returns (psum tile, col slice)."""
                jj = j - g * 4
                vst = max(jj, 0) * P
                vsl = slice(vst, W)
                sps = pmm.tile([P, W], F32, tag="mm", bufs=3, name="sps")
                if jj >= 0:
                    nc.tensor.matmul(sps[:, vsl], id_s, mask_s[:, 0:W - vst],
                                     start=True, stop=False)
                    nc.tensor.matmul(sps[:, vsl],
                                     kT[:, j * P:(j + 1) * P],
                                     qT[:, h, g * W + vst:(g + 1) * W],
                                     start=False, stop=True)
                else:
                    nc.tensor.matmul(sps[:, vsl],
                                     kT[:, j * P:(j + 1) * P],
                                     qT[:, h, gsl],
                                     start=True, stop=True)
                return sps, vsl

            def emit_C(gg):
                """Output projection for tq blocks of group gg."""
                last = (gg == G - 1)
                for i in range(4 * gg, 4 * gg + 4):
                    ost = outp.tile([P, C], F32, tag="ost", name="ost")
                    for cc in range(4):
                        csl = slice(cc * W, (cc + 1) * W)
                        ops = pmm.tile([P, W], F32, tag="mm", bufs=3,
                                       name="ops")
                        for hb in range(NQ):
                            nc.tensor.matmul(ops,
                                             yT[:, hb, i * P:(i + 1) * P],
                                             wpall[:, hb, csl],
                                             start=(hb == 0),
                                             stop=(hb == NQ - 1))
                        nc.scalar.copy(out=ost[:, csl], in_=ops)
                        if last and i == 4 * gg + 3:
                            # final row-block: store per-cc to shrink tail
                            nc.sync.dma_start(
                                out=out[i * P:(i + 1) * P, csl],
                                in_=ost[:, csl])
                    if not (last and i == 4 * gg + 3):
                        # alternate store queues to halve the backlog
                        eng = nc.sync if i % 2 == 0 else nc.gpsimd
                        eng.dma_start(out=out[i * P:(i + 1) * P, :],
                                      in_=ost)

            # ---- phase B: attention (transposed flash, pipelined) ----
            for g in range(G):
                gsl = slice(g * W, (g + 1) * W)
                nblk = 4 * (g + 1)   # causal: tk blocks 0..4(g+1)-1
                for h in range(NQ):
                    yps = pacc.tile([P, W], F32, tag="acc", bufs=3,
                                    name="yps")
                    lps = plps.tile([1, W], F32, tag="lps", bufs=2,
                                    name="lps")
                    pend = [emit_S(g, h, 0, gsl)]
                    if nblk > 1:
                        pend.append(emit_S(g, h, 1, gsl))
                    for j in range(nblk):
                        if j + 2 < nblk:
                            pend.append(emit_S(g, h, j + 2, gsl))
                        sps, vsl = pend[j]
                        pt = ptp.tile([P, W], MM, tag="pt", name="pt")
                        nc.scalar.activation(out=pt[:, vsl], in_=sps[:, vsl],
                                             func=EXP)
                        nc.tensor.matmul(yps[:, vsl], Vt[:, j, :], pt[:, vsl],
                                         start=(j == 0), stop=(j == nblk - 1))
                        nc.tensor.matmul(lps[:, vsl], ones_s, pt[:, vsl],
                                         start=(j == 0), stop=(j == nblk - 1))
                    # normalize: yT[:, h, gsl] = yps * (1/L)[bcast over hd]
                    # 1/L on the [1,W] row. Must NOT use scalar ACTs here:
                    # a non-Exp func forces a ~1.5us activation-table
                    # reload that breaks the exp pipeline.
                    lsb = small.tile([1, W], F32, tag="lsb", name="lsb")
                    nc.vector.reciprocal(out=lsb, in_=lps)
                    rd = dramp.tile([1, W], F32, tag="rd", name="rd")
                    nc.gpsimd.dma_start(out=rd, in_=lsb)
                    rb = stage.tile([P, W], F32, tag="rb", name="rb")
                    nc.gpsimd.dma_start(
                        out=rb,
                        in_=bass.AP(tensor=rd.tensor, offset=rd.offset,
                                    ap=[[0, P]] + [list(d) for d in rd.ap[1:]]))
                    nc.vector.tensor_mul(out=yT[:, h, gsl], in0=yps, in1=rb)
                if g > 0:
                    emit_C(g - 1)
            emit_C(G - 1)
            ctx_bc.close()

    nc.compile()
    return nc


def _phase_a_group(nc, tc, pa, stage, x_t, t, lhs_a, qT, kT, Vt, id_s,
                   rope_apply):
    tsl = slice(t * W, (t + 1) * W)
    for m in range(6):
        ps = pa.tile([P, W], F32, tag="psA", bufs=3, name="ps")
        for c in range(CB):
            nc.tensor.matmul(ps, lhs_a(m, c), x_t[:, c, :],
                             start=(c == 0), stop=(c == CB - 1))
        if m < 4:
            praw = stage.tile([P, W], F32, tag="raw", bufs=6, name="praw")
            nc.scalar.copy(out=praw, in_=ps)
            rope_apply(qT[:, m, tsl], praw, tsl)
        elif m == 4:
            praw = stage.tile([P, W], F32, tag="raw", bufs=6, name="praw")
            nc.scalar.copy(out=praw, in_=ps)
            rope_apply(kT[:, tsl], praw, tsl)
        else:
            vraw = stage.tile([P, W], MM, tag="vraw", name="vraw")
            nc.vector.tensor_copy(out=vraw, in_=ps)
            for jj in range(4):
                j = t * 4 + jj
                pvt = pa.tile([P, P], MM, tag="pvt", bufs=2, name="pvt")
                nc.tensor.transpose(pvt, vraw[:, jj * P:(jj + 1) * P], id_s)
                nc.vector.tensor_copy(out=Vt[:, j, :], in_=pvt)


def make_tables():
    inv = (10000.0 ** (-(np.arange(64, dtype=np.float32) / np.float32(64.0)))
           ).astype(np.float32)
    freqs = np.arange(T, dtype=np.float32)[:, None] * inv[None, :]   # [T, 64]
    cos64 = np.cos(freqs).T.astype(np.float32)                       # [64, T]
    sin64 = np.sin(freqs).T.astype(np.float32)
    cosT = np.concatenate([cos64, cos64], axis=0)                    # [128, T]
    msinT = np.concatenate([-sin64, sin64], axis=0)
    # [P, W] mask table: cols 0..127 = causal triangle for the diagonal
    # 128-block (row=tk-in-block, col=tq-in-block), cols 128.. = 0.
    mask = np.zeros((P, W), dtype=np.float32)
    mask[:, :P] = np.where(
        np.arange(P)[:, None] <= np.arange(P)[None, :],
        np.float32(0.0), np.float32(-1e5))
    ident = np.eye(P, dtype=np.float32)
    return cosT, msinT, mask, ident


def _rearr(a, p=P):
    """[R, M] with R = n*p -> [p, n*M] so each partition line is
    contiguous in DRAM: out[pp, n*M + m] = a[n*p + pp, m]."""
    R, M = a.shape
    n = R // p
    return np.ascontiguousarray(
        a.reshape(n, p, M).transpose(1, 0, 2).reshape(p, n * M))


def _rearr_x(xT):
    """xT [C, T] -> [P, G*CB*W], t-group major: out[p, ((t*CB)+cb)*W + w]
    = xT[cb*P + p, t*W + w], so each (p, t) line is CB*W contiguous."""
    a = xT.reshape(CB, P, G, W).transpose(1, 2, 0, 3)
    return np.ascontiguousarray(a.reshape(P, G * CB * W))


def shard_inputs(x, Wq, Wk, Wv, Wproj):
    import ml_dtypes
    bf16 = ml_dtypes.bfloat16
    cosT, msinT, mask, ident = make_tables()
    scale = np.float32(1.0 / np.sqrt(np.float32(HD)))
    xRb = [_rearr_x(np.ascontiguousarray(x[b].T)).astype(bf16)
           for b in range(B)]
    in_maps = []
    for core in range(8):
        b, g = core // 4, core % 4
        in_maps.append({
            "xR": xRb[b],
            "wqr": _rearr(Wq[:, g * NQ * HD:(g + 1) * NQ * HD] * scale
                          ).astype(bf16),
            "wkr": _rearr(Wk[:, g * HD:(g + 1) * HD]).astype(bf16),
            "wvr": _rearr(Wv[:, g * HD:(g + 1) * HD]).astype(bf16),
            "wpr": _rearr(Wproj[g * NQ * HD:(g + 1) * NQ * HD, :]).astype(bf16),
            "cosT": cosT.astype(bf16), "msinT": msinT.astype(bf16),
            "masktb": mask.astype(bf16),
            "ident": ident.astype(bf16),
            "onescol": np.ones((P, 1), dtype=bf16),
        })
    return in_maps


_NC_CACHE = {}


def _get_nc():
    key = USE_F32R
    if key not in _NC_CACHE:
        _NC_CACHE[key] = build_nc()
    return _NC_CACHE[key]


def kernel(x, Wq, Wk, Wv, Wproj):
    from concourse.bass_utils import run_bass_kernel_spmd
    x = np.asarray(x, dtype=np.float32)
    Wq = np.asarray(Wq, dtype=np.float32)
    Wk = np.asarray(Wk, dtype=np.float32)
    Wv = np.asarray(Wv, dtype=np.float32)
    Wproj = np.asarray(Wproj, dtype=np.float32)
    nc = _get_nc()
    in_maps = shard_inputs(x, Wq, Wk, Wv, Wproj)
    res = run_bass_kernel_spmd(nc, in_maps, core_ids=list(range(8)))
    out = np.zeros((B, T, C), dtype=np.float32)
    for core in range(8):
        b = core // 4
        out[b] += res.results[core]["out"]
    return out

